# revision 10
# baseline (speedup 1.0000x reference)
"""GAT-style attentive layer on 8 TRN2 NeuronCores — fp8 DoubleRow version.

Math (per reference):
    Wh  = input                      [N, D]   (N=8192, D=512)
    Wh1 = Wh @ a[:D]  (s_i)          [N, 1]
    Wh2 = Wh @ a[D:]  (t_j)          [N, 1]
    e   = leaky_relu(Wh1 + Wh2.T, 0.01)
    e   = where(adj > 0, e, -9e15)
    att = softmax(e, axis=1)
    out = att @ Wh                   [N, D]

Sharding: row-shard the N x N attention across 8 cores (1024 rows each).
Scores are produced transposed, pT[j, i] (j on partitions), so the final
matmul uses pT tiles as the stationary operand.

Key speed tricks (all validated against the TimelineSim cost model):
 - The big matmul runs in fp8e4 (e4m3) with MatmulPerfMode.DoubleRow: one
   PE instruction contracts TWO 128-deep j-tiles at 0.5 cycles per output
   column (4x the bf16 rate per MAC).  Accuracy is held by splitting Wh
   into hi+lo e4m3 planes (w = w_hi + w_lo reconstructs bf16-level w), so
   only the attention weights p carry fp8 noise; measured end-to-end
   rel-err 1.66e-2 against the f32 oracle (gate: 2e-2).
 - p8 values are produced by a single Act pass per score tile: the
   compiler's Exp table is patched to compute exp(lrelu(x) - 1.25), with
   x <= -20 mapped to exact 0.  The -1.25 shift keeps p below e4m3's 240
   max (softmax cancels the shift), lrelu is fused into the table, and
   the adjacency mask is additive ({0, -112} fp8), folded in by the
   score-add.
 - The score-add s_t = (adjT8 + t_j) + bcast_wh1 is a single
   scalar_tensor_tensor op (per-partition t_j rides the scalar slot),
   split between the DVE and gpsimd engines to keep both under the Act
   engine's throughput.  Folding t_j here (instead of the Act bias) lets
   the Act pass batch 4 j-tiles per instruction.
 - Row sums ride a third DoubleRow matmul per pair against a ones pair
   (~1 cycle each); normalization is a reciprocal-multiply on the DVE.

Host-side prep (data marshaling only): dtype casts + transpose/slicing;
all compute (projections, scores, exp, matmul, normalize) runs on device.
"""

import os

import numpy as np
import ml_dtypes

import concourse.bass as bass
import concourse.mybir as mybir
import concourse.tile as tile
from concourse import bacc
from concourse.bass_utils import run_bass_kernel_spmd

N = 8192          # nodes
D = 512           # feature dim
NCORES = 8
ROWS = N // NCORES  # 1024 output rows per core
P = 128
NJT = N // P      # 64 j-tiles
NPAIR = NJT // 2  # 32 j-tile pairs (one DoubleRow contraction each)
IC_W = 512        # i-chunk width (PSUM-limited)
NIC = ROWS // IC_W  # 2 i-chunks
ITPC = IC_W // P  # 4 i-subtiles per chunk

MASK_NEG = -112.0  # additive mask (e4m3-exact); table maps x <= -20 to 0
C_SHIFT = 1.25     # table computes exp(lrelu(x) - C_SHIFT): keeps p < 240

AF = mybir.ActivationFunctionType
ALU = mybir.AluOpType
PM = mybir.MatmulPerfMode
dt = mybir.dt
F32 = dt.float32
BF16 = dt.bfloat16
FP8 = dt.float8e4

# All score-adds run on the DVE: the per-partition t_j scalar rides the
# TensorScalarPtr opcode, which the Pool engine does not implement.
def _add_on_pool(jt: int, ic: int) -> bool:
    return False


def _build_kernel(nc: bass.Bass, tc: tile.TileContext,
                  adjT8: bass.AP, whi: bass.AP, wlo: bass.AP,
                  xT: bass.AP, a8t_d: bass.AP, out: bass.AP, ctx):
    pool_const = ctx.enter_context(tc.tile_pool(name="const", bufs=1))
    pool_w = ctx.enter_context(tc.tile_pool(name="w", bufs=1))
    pool_adj = ctx.enter_context(tc.tile_pool(name="adj", bufs=8))
    pool_st = ctx.enter_context(tc.tile_pool(name="st", bufs=3))
    pool_p8 = ctx.enter_context(tc.tile_pool(name="p8", bufs=3))
    pool_outs = ctx.enter_context(tc.tile_pool(name="outs", bufs=2))
    pool_small = ctx.enter_context(tc.tile_pool(name="small", bufs=1))
    pool_psum = ctx.enter_context(tc.tile_pool(name="psum", bufs=1, space="PSUM"))

    # ---- constants / warmup ------------------------------------------------
    warm = pool_const.tile([1, 2], F32, tag="warm", name="warm")
    nc.vector.memset(warm, 0.0)
    nc.scalar.activation(warm, warm, AF.Exp)  # pull ACT_TABLE_LOAD to t~0

    ones2 = pool_const.tile([P, 2, 2], FP8, tag="ones2", name="ones2")
    nc.vector.memset(ones2, 1.0)

    a8t = pool_const.tile([P, 8], BF16, tag="a8t", name="a8t")
    nc.sync.dma_start(a8t, a8t_d)

    # ---- xT (transposed x, bf16) for wh1 + wh2 projections -----------------
    # Streamed in 8 j-range chunks of [128, 4, 1024] (1MB) through a 3-buf
    # ring, so t-projections for early j-tiles unblock fast and the full
    # 8MB never sits in SBUF.  The host rotates j per core so chunk 0 is
    # always this core's own rows (which also serve wh1).
    pool_xT = ctx.enter_context(tc.tile_pool(name="xT", bufs=3))
    xTc = {}

    def dma_xT_chunk(g):
        t = pool_xT.tile([P, 4, ROWS], BF16, tag="xTc", name="xTc", bufs=3)
        nc.sync.dma_start(
            t, xT[:, bass.ds(g * ROWS, ROWS)].rearrange("(t p) j -> p t j", p=P))
        xTc[g] = t

    dma_xT_chunk(0)

    bcast16 = [pool_const.tile([P, IC_W], BF16, tag=f"bw{h}", name=f"bw{h}")
               for h in range(NIC)]
    wh1_row = pool_const.tile([1, ROWS], BF16, tag="w1r", name="w1r")

    def wh1_compute(h):
        ps = pool_psum.tile([1, IC_W], F32, tag="w1p", name="w1p", bufs=1)
        for t in range(4):
            nc.tensor.matmul(ps, lhsT=a8t[:, t:t + 1],
                             rhs=xTc[0][:, t, bass.ds(h * IC_W, IC_W)],
                             start=(t == 0), stop=(t == 3),
                             skip_group_check=True)
        nc.scalar.copy(wh1_row[:, bass.ds(h * IC_W, IC_W)], ps)
        nc.gpsimd.partition_broadcast(bcast16[h],
                                      wh1_row[0:1, bass.ds(h * IC_W, IC_W)])

    # t_j for ALL j (wh2), in bias-column layout [128, 64]: per j-tile a
    # [128, 1] psum column from 4 k-matmuls (ap_size=1: nearly free on PE).
    # One shared PSUM bank, memset once, all matmuls start=False (start=True
    # would zero the whole bank and wipe earlier columns).
    tcol_ps = pool_psum.tile([P, NJT], F32, tag="tcol", name="tcol")
    nc.vector.memset(tcol_ps, 0.0)
    wh2_sb = pool_const.tile([P, NJT], F32, tag="wh2", name="wh2")

    def t_compute(g):
        # group g covers j-tiles 8g..8g+7 (chunk g's 1024 columns)
        for k in range(8):
            jt = 8 * g + k
            for t in range(4):
                nc.tensor.matmul(tcol_ps[:, jt:jt + 1],
                                 lhsT=xTc[g][:, t, bass.ds(k * P, P)],
                                 rhs=a8t[:, 4 + t:5 + t],
                                 start=False, stop=(t == 3),
                                 skip_group_check=True)
        nc.scalar.copy(wh2_sb[:, bass.ds(8 * g, 8)], tcol_ps[:, bass.ds(8 * g, 8)])
        del xTc[g]

    wh1_compute(0)
    wh1_compute(1)
    t_compute(0)
    dma_xT_chunk(1)

    # ---- W planes (resident) + adj pair-tile DMAs --------------------------
    whq_hi, whq_lo = [], []
    adj_pre = {}

    def dma_adj_pair(pair, ic):
        t = pool_adj.tile([P, 2, IC_W], FP8, tag="adjq", name="adjq", bufs=20)
        nc.sync.dma_start(
            t, adjT8[bass.ds(pair * 2 * P, 2 * P),
                     bass.ds(ic * IC_W, IC_W)].rearrange("(q p) i -> p q i", p=P))
        return t

    # Interleave: W planes + early adj pairs + xT chunks, in consumption
    # order.  Only the first 10 adj pairs load here; the rest prefetch
    # inside the main loop to bound SBUF and keep the DMA queue fluid.
    for m in range(16):
        thi = pool_w.tile([P, 4, D], FP8, tag=f"whi{m}", name=f"whi{m}")
        nc.sync.dma_start(thi, whi[bass.ds(m * 4 * P, 4 * P), :]
                          .rearrange("(q p) d -> p q d", p=P))
        whq_hi.append(thi)
        tlo = pool_w.tile([P, 4, D], FP8, tag=f"wlo{m}", name=f"wlo{m}")
        nc.sync.dma_start(tlo, wlo[bass.ds(m * 4 * P, 4 * P), :]
                          .rearrange("(q p) d -> p q d", p=P))
        whq_lo.append(tlo)
        if m < 5:
            adj_pre[(2 * m, 0)] = dma_adj_pair(2 * m, 0)
            adj_pre[(2 * m + 1, 0)] = dma_adj_pair(2 * m + 1, 0)
        if m == 0:
            t_compute(1)
        elif m <= 6:
            dma_xT_chunk(m + 1)
            t_compute(m + 1)

    # ---- row-sum PSUM banks (pre-zeroed; accumulate with start=False) ------
    rsb = []
    for ic in range(NIC):
        t = pool_psum.tile([P, 2 * ITPC], F32, tag=f"prs{ic}", name=f"prs{ic}")
        nc.vector.memset(t, 0.0)
        rsb.append(t)

    # ---- main loop ---------------------------------------------------------
    for ic in range(NIC):
        psum_out = [
            pool_psum.tile([P, D], F32, tag=f"po{i}", name=f"po{i}")
            for i in range(ITPC)
        ]
        for half in range(NPAIR // 2):  # 16 quad-groups of 4 j-tiles
            quad = half  # quad index: j-tiles 4*quad .. 4*quad+3
            s_q = pool_st.tile([P, 4, IC_W], BF16, tag="s_q", name="s_q")
            for k in range(4):
                jt = 4 * quad + k
                seng = nc.gpsimd if _add_on_pool(jt, ic) else nc.vector
                pair = jt // 2
                if (pair, ic) in adj_pre:
                    adjq = adj_pre.pop((pair, ic))
                # staggered adj prefetches (consumption order, ~5-8 quads
                # of lead, ≤ ~20 tiles in flight)
                if ic == 0 and k < 2 and 10 + 2 * quad + k <= 31:
                    pf = 10 + 2 * quad + k
                    adj_pre[(pf, 0)] = dma_adj_pair(pf, 0)
                if ic == 0 and quad >= 8 and k >= 2:
                    pf = 2 * (quad - 8) + (k - 2)
                    adj_pre[(pf, 1)] = dma_adj_pair(pf, 1)
                if ic == 1 and k < 2 and 16 + 2 * quad + k <= 31:
                    pf = 16 + 2 * quad + k
                    adj_pre[(pf, 1)] = dma_adj_pair(pf, 1)
                seng.scalar_tensor_tensor(
                    out=s_q[:, k, :], in0=adjq[:, jt % 2, :],
                    scalar=wh2_sb[:, jt:jt + 1], in1=bcast16[ic],
                    op0=ALU.add, op1=ALU.add)
            p8 = pool_p8.tile([P, 4, IC_W], FP8, tag="p8", name="p8")
            nc.scalar.activation(p8, s_q, AF.Exp)

            for pp in range(2):  # the 2 pairs inside this quad
                pair = 2 * quad + pp
                first = pair == 0
                last = pair == NPAIR - 1
                for i4 in range(ITPC):
                    lhs = p8[:, bass.ds(2 * pp, 2), bass.ds(i4 * P, P)]
                    nc.tensor.matmul(psum_out[i4], lhsT=lhs,
                                     rhs=whq_hi[pair // 2][:, bass.ds(2 * (pair % 2), 2), :],
                                     start=first, stop=False,
                                     perf_mode=PM.DoubleRow,
                                     skip_group_check=not first)
                    nc.tensor.matmul(psum_out[i4], lhsT=lhs,
                                     rhs=whq_lo[pair // 2][:, bass.ds(2 * (pair % 2), 2), :],
                                     start=False, stop=last,
                                     perf_mode=PM.DoubleRow,
                                     skip_group_check=True)
                    nc.tensor.matmul(rsb[ic][:, 2 * i4:2 * i4 + 2], lhsT=lhs,
                                     rhs=ones2,
                                     start=False, stop=last,
                                     perf_mode=PM.DoubleRow,
                                     skip_group_check=True)

        # ---- normalize + ship ----------------------------------------------
        outq = pool_outs.tile([P, ITPC, D], BF16, tag="outq", name="outq",
                              bufs=2)
        recip8 = pool_small.tile([P, 2 * ITPC], F32, tag="recip", name="recip",
                                 bufs=2)
        nc.vector.reciprocal(recip8, rsb[ic])
        last_ic = ic == NIC - 1
        for i4 in range(ITPC):
            recip = recip8[:, 2 * i4:2 * i4 + 1]
            if last_ic and i4 % 2 == 1:
                nc.scalar.mul(outq[:, i4, :], psum_out[i4], recip)
            else:
                nc.vector.tensor_scalar_mul(outq[:, i4, :], psum_out[i4], recip)
            if last_ic and i4 == 1:
                nc.sync.dma_start(
                    out[bass.ds(ic * IC_W, 2 * P), :].rearrange(
                        "(q p) d -> p q d", p=P), outq[:, 0:2, :])
        if last_ic:
            nc.sync.dma_start(
                out[bass.ds(ic * IC_W + 2 * P, 2 * P), :].rearrange(
                    "(q p) d -> p q d", p=P), outq[:, 2:4, :])
        else:
            nc.sync.dma_start(
                out[bass.ds(ic * IC_W, IC_W), :].rearrange(
                    "(q p) d -> p q d", p=P), outq)


_CACHED = None

_FUSED_ALPHA = 0.01
_ZERO_BELOW = -20.0  # table inputs below this produce exact 0


def _make_fused_act_root() -> str:
    """Copy the compiler's activation-table dir, patching Exp to compute
      x < -20:       exactly 0 (additively-masked scores exp to zero)
      x in [-20, 0): exp(_FUSED_ALPHA*x - C_SHIFT)   (lrelu fused)
      x >= 0:        exp(x - C_SHIFT)
    The -C_SHIFT keeps outputs under e4m3's 240 max; softmax cancels it.
    Returns path to the patched act_info.json."""
    import json
    import shutil
    import tempfile

    from neuronxcc.driver.Job import Job
    from neuronxcc.driver.jobs.support.FindActInfo import findActInfoFile

    src_root = os.path.dirname(findActInfoFile(Job.getPackageDir(), "gen3"))
    dst = tempfile.mkdtemp(prefix="act_root_fused_")
    for f in os.listdir(src_root):
        shutil.copy(os.path.join(src_root, f), os.path.join(dst, f))
    info = json.load(open(os.path.join(dst, "act_info.json")))
    scale = np.float64(np.exp(-C_SHIFT))
    for s in info["act_func_sets"]:
        if "exp" not in s["act"]:
            continue
        prof = json.load(open(os.path.join(dst, s["profile_json"])))
        order = sorted(prof["func_to_bkt_start_idx"].items(), key=lambda kv: kv[1])
        idx = [i for i, (k, _) in enumerate(order) if k == "exp"][0]
        lo = order[idx][1]
        hi = order[idx + 1][1] if idx + 1 < len(order) else prof["bkt_entry_cnt"]
        path = os.path.join(dst, s["bkt_bin"])
        bkt = np.fromfile(path, dtype=np.float32).reshape(-1, 8).copy()
        for b in range(lo, hi):
            d0, d1, _, _, x0 = bkt[b, :5]
            if x0 <= _ZERO_BELOW:
                bkt[b, 0:4] = 0.0  # masked region: exp -> exact 0
                continue
            if not (d0 > 0 and np.isfinite(d0) and abs(d1 - d0) <= 1e-3 * d0):
                continue  # saturation buckets (inf / 0)
            if x0 > 0:
                # positive side: exp(x - C)
                g = np.float32(np.exp(np.float64(x0) - C_SHIFT))
                bkt[b, 0] = g
                bkt[b, 1] = g
            else:
                # negative side: exp(alpha*x - C) (nearly flat; linear spline)
                g = np.float32(np.exp(_FUSED_ALPHA * np.float64(x0) - C_SHIFT))
                bkt[b, 0] = g
                bkt[b, 1] = np.float32(_FUSED_ALPHA * g)
            bkt[b, 2] = np.float32(0.0)  # cubic terms fault the engine
            bkt[b, 3] = np.float32(0.0)
        bkt.tofile(path)
    return os.path.join(dst, "act_info.json")


def build_nc():
    global _CACHED
    if _CACHED is not None:
        return _CACHED
    os.environ["BASS_ACT_ROOT_JSON_PATH"] = _make_fused_act_root()
    nc = bacc.Bacc("TRN2", target_bir_lowering=False, debug=False,
                   enable_asserts=False, num_devices=NCORES)
    adjT8 = nc.dram_tensor("adjT8", [N, ROWS], FP8, kind="ExternalInput").ap()
    whi = nc.dram_tensor("whi", [N, D], FP8, kind="ExternalInput").ap()
    wlo = nc.dram_tensor("wlo", [N, D], FP8, kind="ExternalInput").ap()
    xT = nc.dram_tensor("xT", [D, N], BF16, kind="ExternalInput").ap()
    a8t = nc.dram_tensor("a8t", [P, 8], BF16, kind="ExternalInput").ap()
    out = nc.dram_tensor("out", [ROWS, D], BF16, kind="ExternalOutput").ap()

    from contextlib import ExitStack
    with tile.TileContext(nc) as tc:
        with ExitStack() as ctx:
            _build_kernel(nc, tc, adjT8, whi, wlo, xT, a8t, out, ctx)
    nc.compile()
    _CACHED = nc
    return nc


def make_in_maps(input, adj_matrix, a):
    E4 = ml_dtypes.float8_e4m3
    BF = ml_dtypes.bfloat16
    x16 = np.asarray(input, dtype=np.float32).astype(BF)
    x16f = x16.astype(np.float32)
    w_hi = np.ascontiguousarray(x16f.astype(E4))
    w_lo = np.ascontiguousarray((x16f - w_hi.astype(np.float32)).astype(E4))
    adj = np.asarray(adj_matrix)
    a_f = np.asarray(a, dtype=np.float32).reshape(-1)
    a8t = np.ascontiguousarray(a_f.reshape(8, P).T.astype(BF))  # [128, 8]
    xT_full = np.ascontiguousarray(x16.T)                        # [D, N] bf16
    in_maps = []
    for c in range(NCORES):
        rows = slice(c * ROWS, (c + 1) * ROWS)
        # per-core j-rotation: tile 0 is always this core's own rows
        rot = np.roll(np.arange(N), -c * ROWS)
        adjT_c = adj[rows, :].T[rot]          # [N(j rotated), ROWS(i local)]
        adjT8_c = np.ascontiguousarray(
            ((adjT_c.astype(np.float32) - 1.0) * (-MASK_NEG)).astype(E4))
        in_maps.append({
            "adjT8": adjT8_c,
            "whi": np.ascontiguousarray(w_hi[rot]),
            "wlo": np.ascontiguousarray(w_lo[rot]),
            "xT": np.ascontiguousarray(xT_full[:, rot]),
            "a8t": a8t,
        })
    return in_maps


def kernel(input, adj_matrix, a, _trace=False, _tmpdir=None):
    nc = build_nc()
    in_maps = make_in_maps(input, adj_matrix, a)
    try:
        res = run_bass_kernel_spmd(nc, in_maps, core_ids=list(range(NCORES)),
                                   trace=_trace, tmpdir=_tmpdir)
    except ModuleNotFoundError:
        res = run_bass_kernel_spmd(nc, in_maps, core_ids=list(range(NCORES)))
    out = np.concatenate(
        [res.results[c]["out"].astype(np.float32) for c in range(NCORES)],
        axis=0)
    kernel._last_results = res
    return out


# revision 15
# speedup vs baseline: 1.1565x; 1.1565x over previous
"""GAT-style attentive layer on 8 TRN2 NeuronCores — fp8 DoubleRow version.

Math (per reference):
    Wh  = input                      [N, D]   (N=8192, D=512)
    Wh1 = Wh @ a[:D]  (s_i)          [N, 1]
    Wh2 = Wh @ a[D:]  (t_j)          [N, 1]
    e   = leaky_relu(Wh1 + Wh2.T, 0.01)
    e   = where(adj > 0, e, -9e15)
    att = softmax(e, axis=1)
    out = att @ Wh                   [N, D]

Sharding: row-shard the N x N attention across 8 cores (1024 rows each).
Scores are produced transposed, pT[j, i] (j on partitions), so the final
matmul uses pT tiles as the stationary operand.

Key speed tricks (all validated against the TimelineSim cost model):
 - The big matmul runs in fp8e4 (e4m3) with MatmulPerfMode.DoubleRow: one
   PE instruction contracts TWO 128-deep j-tiles at 0.5 cycles per output
   column (4x the bf16 rate per MAC).  Accuracy is held by splitting Wh
   into hi+lo e4m3 planes (w = w_hi + w_lo reconstructs bf16-level w), so
   only the attention weights p carry fp8 noise; measured end-to-end
   rel-err 1.66e-2 against the f32 oracle (gate: 2e-2).
 - p8 values are produced by a single Act pass per score tile: the
   compiler's Exp table is patched to compute exp(lrelu(x) - 1.25), with
   x <= -20 mapped to exact 0.  The -1.25 shift keeps p below e4m3's 240
   max (softmax cancels the shift), lrelu is fused into the table, and
   the adjacency mask is additive ({0, -112} fp8), folded in by the
   score-add.
 - The score-add s_t = (adjT8 + t_j) + bcast_wh1 is a single
   scalar_tensor_tensor op (per-partition t_j rides the scalar slot),
   split between the DVE and gpsimd engines to keep both under the Act
   engine's throughput.  Folding t_j here (instead of the Act bias) lets
   the Act pass batch 4 j-tiles per instruction.
 - Row sums ride a third DoubleRow matmul per pair against a ones pair
   (~1 cycle each); normalization is a reciprocal-multiply on the DVE.

Host-side prep (data marshaling only): dtype casts + transpose/slicing;
all compute (projections, scores, exp, matmul, normalize) runs on device.
"""

import os

import numpy as np
import ml_dtypes

import concourse.bass as bass
import concourse.mybir as mybir
import concourse.tile as tile
from concourse import bacc
from concourse.bass_utils import run_bass_kernel_spmd

N = 8192          # nodes
D = 512           # feature dim
NCORES = 8
ROWS = N // NCORES  # 1024 output rows per core
P = 128
NJT = N // P      # 64 j-tiles
NPAIR = NJT // 2  # 32 j-tile pairs (one DoubleRow contraction each)
IC_W = 512        # i-chunk width (PSUM-limited)
NIC = ROWS // IC_W  # 2 i-chunks
ITPC = IC_W // P  # 4 i-subtiles per chunk

MASK_NEG = -112.0  # additive mask (e4m3-exact); table maps x <= -20 to 0
C_SHIFT = 1.25     # table computes exp(lrelu(x) - C_SHIFT): keeps p < 240

AF = mybir.ActivationFunctionType
ALU = mybir.AluOpType
PM = mybir.MatmulPerfMode
dt = mybir.dt
F32 = dt.float32
BF16 = dt.bfloat16
FP8 = dt.float8e4

# All score-adds run on the DVE: the per-partition t_j scalar rides the
# TensorScalarPtr opcode, which the Pool engine does not implement.
def _add_on_pool(jt: int, ic: int) -> bool:
    return False


def _build_kernel(nc: bass.Bass, tc: tile.TileContext,
                  adjT8: bass.AP, whi: bass.AP, wlo: bass.AP,
                  xT: bass.AP, a8t_d: bass.AP, out: bass.AP, ctx):
    pool_const = ctx.enter_context(tc.tile_pool(name="const", bufs=1))
    pool_w = ctx.enter_context(tc.tile_pool(name="w", bufs=1))
    pool_adj = ctx.enter_context(tc.tile_pool(name="adj", bufs=8))
    pool_st = ctx.enter_context(tc.tile_pool(name="st", bufs=3))
    pool_p8 = ctx.enter_context(tc.tile_pool(name="p8", bufs=3))
    pool_outs = ctx.enter_context(tc.tile_pool(name="outs", bufs=2))
    pool_small = ctx.enter_context(tc.tile_pool(name="small", bufs=1))
    pool_psum = ctx.enter_context(tc.tile_pool(name="psum", bufs=1, space="PSUM"))

    # ---- constants / warmup ------------------------------------------------
    warm = pool_const.tile([1, 2], F32, tag="warm", name="warm")
    nc.vector.memset(warm, 0.0)
    nc.scalar.activation(warm, warm, AF.Exp)  # pull ACT_TABLE_LOAD to t~0

    ones2 = pool_const.tile([P, 2, 2], FP8, tag="ones2", name="ones2")
    nc.vector.memset(ones2, 1.0)

    a8t = pool_const.tile([P, 8], BF16, tag="a8t", name="a8t")
    nc.sync.dma_start(a8t, a8t_d)

    # ---- xT (transposed x, bf16) for wh1 + wh2 projections -----------------
    # Streamed in 8 j-range chunks of [128, 4, 1024] (1MB) through a 3-buf
    # ring, so t-projections for early j-tiles unblock fast and the full
    # 8MB never sits in SBUF.  The host rotates j per core so chunk 0 is
    # always this core's own rows (which also serve wh1).
    pool_xT = ctx.enter_context(tc.tile_pool(name="xT", bufs=3))
    xTc = {}

    def dma_xT_chunk(g):
        t = pool_xT.tile([P, 4, ROWS], BF16, tag="xTc", name="xTc", bufs=3)
        nc.sync.dma_start(
            t, xT[:, bass.ds(g * ROWS, ROWS)].rearrange("(t p) j -> p t j", p=P))
        xTc[g] = t

    dma_xT_chunk(0)
    # (a8t DMA already queued above; chunk 1 queued right after wh1/t(0)
    # below so the queue starts with the critical prologue bytes.)

    bcast16 = [pool_const.tile([P, IC_W], BF16, tag=f"bw{h}", name=f"bw{h}")
               for h in range(NIC)]
    wh1_row = pool_const.tile([1, ROWS], BF16, tag="w1r", name="w1r")

    def wh1_compute(h):
        ps = pool_psum.tile([1, IC_W], F32, tag="w1p", name="w1p", bufs=1)
        for t in range(4):
            nc.tensor.matmul(ps, lhsT=a8t[:, t:t + 1],
                             rhs=xTc[0][:, t, bass.ds(h * IC_W, IC_W)],
                             start=(t == 0), stop=(t == 3),
                             skip_group_check=True)
        nc.scalar.copy(wh1_row[:, bass.ds(h * IC_W, IC_W)], ps)
        nc.gpsimd.partition_broadcast(bcast16[h],
                                      wh1_row[0:1, bass.ds(h * IC_W, IC_W)])

    # t_j for ALL j (wh2), in bias-column layout [128, 64]: per j-tile a
    # [128, 1] psum column from 4 k-matmuls (ap_size=1: nearly free on PE).
    # One shared PSUM bank, memset once, all matmuls start=False (start=True
    # would zero the whole bank and wipe earlier columns).
    tcol_ps = pool_psum.tile([P, NJT], F32, tag="tcol", name="tcol")
    nc.vector.memset(tcol_ps, 0.0)
    wh2_sb = pool_const.tile([P, NJT], F32, tag="wh2", name="wh2")

    def t_compute(g):
        # group g covers j-tiles 8g..8g+7 (chunk g's 1024 columns)
        for k in range(8):
            jt = 8 * g + k
            for t in range(4):
                nc.tensor.matmul(tcol_ps[:, jt:jt + 1],
                                 lhsT=xTc[g][:, t, bass.ds(k * P, P)],
                                 rhs=a8t[:, 4 + t:5 + t],
                                 start=False, stop=(t == 3),
                                 skip_group_check=True)
        nc.scalar.copy(wh2_sb[:, bass.ds(8 * g, 8)], tcol_ps[:, bass.ds(8 * g, 8)])
        del xTc[g]

    wh1_compute(0)
    wh1_compute(1)
    t_compute(0)
    dma_xT_chunk(1)

    # ---- W planes (resident) + adj pair-tile DMAs --------------------------
    whq_hi, whq_lo = [], []
    adj_pre = {}

    def dma_adj_pair(pair, ic):
        t = pool_adj.tile([P, 2, IC_W], FP8, tag="adjq", name="adjq", bufs=40)
        nc.sync.dma_start(
            t, adjT8[bass.ds(pair * 2 * P, 2 * P),
                     bass.ds(ic * IC_W, IC_W)].rearrange("(q p) i -> p q i", p=P))
        return t

    # Interleave W planes + ALL ic0 adj pairs + xT chunks in strict
    # consumption order: pair p is consumed at pipeline-time ~1.23us*p, W
    # group m at pair 2m, chunk g before pair 4g.  The DMA queue is FIFO;
    # order here IS the delivery schedule.
    for m in range(16):
        adj_pre[(2 * m, 0)] = dma_adj_pair(2 * m, 0)
        thi = pool_w.tile([P, 4, D], FP8, tag=f"whi{m}", name=f"whi{m}")
        nc.sync.dma_start(thi, whi[bass.ds(m * 4 * P, 4 * P), :]
                          .rearrange("(q p) d -> p q d", p=P))
        whq_hi.append(thi)
        adj_pre[(2 * m + 1, 0)] = dma_adj_pair(2 * m + 1, 0)
        tlo = pool_w.tile([P, 4, D], FP8, tag=f"wlo{m}", name=f"wlo{m}")
        nc.sync.dma_start(tlo, wlo[bass.ds(m * 4 * P, 4 * P), :]
                          .rearrange("(q p) d -> p q d", p=P))
        whq_lo.append(tlo)
        if m <= 5:
            dma_xT_chunk(m + 2)
        if 1 <= m <= 7:
            t_compute(m)

    # ---- row-sum PSUM banks (pre-zeroed; accumulate with start=False) ------
    rsb = []
    for ic in range(NIC):
        t = pool_psum.tile([P, 2 * ITPC], F32, tag=f"prs{ic}", name=f"prs{ic}")
        nc.vector.memset(t, 0.0)
        rsb.append(t)

    # ---- main loop ---------------------------------------------------------
    for ic in range(NIC):
        psum_out = [
            pool_psum.tile([P, D], F32, tag=f"po{i}", name=f"po{i}")
            for i in range(ITPC)
        ]
        for half in range(NPAIR // 2):  # 16 quad-groups of 4 j-tiles
            quad = half  # quad index: j-tiles 4*quad .. 4*quad+3
            s_q = pool_st.tile([P, 4, IC_W], BF16, tag="s_q", name="s_q")
            for k in range(4):
                jt = 4 * quad + k
                seng = nc.gpsimd if _add_on_pool(jt, ic) else nc.vector
                pair = jt // 2
                if (pair, ic) in adj_pre:
                    adjq = adj_pre.pop((pair, ic))
                # ic1 adj prefetch: 2 per ic0 quad, in consumption order;
                # lands right behind the prologue stream.
                if ic == 0 and k >= 2:
                    pf = 2 * quad + (k - 2)
                    adj_pre[(pf, 1)] = dma_adj_pair(pf, 1)
                seng.scalar_tensor_tensor(
                    out=s_q[:, k, :], in0=adjq[:, jt % 2, :],
                    scalar=wh2_sb[:, jt:jt + 1], in1=bcast16[ic],
                    op0=ALU.add, op1=ALU.add)
            p8 = pool_p8.tile([P, 4, IC_W], FP8, tag="p8", name="p8")
            nc.scalar.activation(p8, s_q, AF.Exp)

            for pp in range(2):  # the 2 pairs inside this quad
                pair = 2 * quad + pp
                first = pair == 0
                last = pair == NPAIR - 1
                for i4 in range(ITPC):
                    lhs = p8[:, bass.ds(2 * pp, 2), bass.ds(i4 * P, P)]
                    nc.tensor.matmul(psum_out[i4], lhsT=lhs,
                                     rhs=whq_hi[pair // 2][:, bass.ds(2 * (pair % 2), 2), :],
                                     start=first, stop=False,
                                     perf_mode=PM.DoubleRow,
                                     skip_group_check=not first)
                    nc.tensor.matmul(psum_out[i4], lhsT=lhs,
                                     rhs=whq_lo[pair // 2][:, bass.ds(2 * (pair % 2), 2), :],
                                     start=False, stop=last,
                                     perf_mode=PM.DoubleRow,
                                     skip_group_check=True)
                    nc.tensor.matmul(rsb[ic][:, 2 * i4:2 * i4 + 2], lhsT=lhs,
                                     rhs=ones2,
                                     start=False, stop=last,
                                     perf_mode=PM.DoubleRow,
                                     skip_group_check=True)

        # ---- normalize + ship ----------------------------------------------
        outq = pool_outs.tile([P, ITPC, D], BF16, tag="outq", name="outq",
                              bufs=2)
        recip8 = pool_small.tile([P, 2 * ITPC], F32, tag="recip", name="recip",
                                 bufs=2)
        nc.vector.reciprocal(recip8, rsb[ic])
        last_ic = ic == NIC - 1
        for i4 in range(ITPC):
            recip = recip8[:, 2 * i4:2 * i4 + 1]
            nc.scalar.mul(outq[:, i4, :], psum_out[i4], recip)
            if last_ic and i4 == 1:
                nc.sync.dma_start(
                    out[bass.ds(ic * IC_W, 2 * P), :].rearrange(
                        "(q p) d -> p q d", p=P), outq[:, 0:2, :])
        if last_ic:
            nc.sync.dma_start(
                out[bass.ds(ic * IC_W + 2 * P, 2 * P), :].rearrange(
                    "(q p) d -> p q d", p=P), outq[:, 2:4, :])
        else:
            nc.sync.dma_start(
                out[bass.ds(ic * IC_W, IC_W), :].rearrange(
                    "(q p) d -> p q d", p=P), outq)


_CACHED = None

_FUSED_ALPHA = 0.01
_ZERO_BELOW = -20.0  # table inputs below this produce exact 0


def _make_fused_act_root() -> str:
    """Copy the compiler's activation-table dir, patching Exp to compute
      x < -20:       exactly 0 (additively-masked scores exp to zero)
      x in [-20, 0): exp(_FUSED_ALPHA*x - C_SHIFT)   (lrelu fused)
      x >= 0:        exp(x - C_SHIFT)
    The -C_SHIFT keeps outputs under e4m3's 240 max; softmax cancels it.
    Returns path to the patched act_info.json."""
    import json
    import shutil
    import tempfile

    from neuronxcc.driver.Job import Job
    from neuronxcc.driver.jobs.support.FindActInfo import findActInfoFile

    src_root = os.path.dirname(findActInfoFile(Job.getPackageDir(), "gen3"))
    dst = tempfile.mkdtemp(prefix="act_root_fused_")
    for f in os.listdir(src_root):
        shutil.copy(os.path.join(src_root, f), os.path.join(dst, f))
    info = json.load(open(os.path.join(dst, "act_info.json")))
    scale = np.float64(np.exp(-C_SHIFT))
    for s in info["act_func_sets"]:
        if "exp" not in s["act"]:
            continue
        prof = json.load(open(os.path.join(dst, s["profile_json"])))
        order = sorted(prof["func_to_bkt_start_idx"].items(), key=lambda kv: kv[1])
        idx = [i for i, (k, _) in enumerate(order) if k == "exp"][0]
        lo = order[idx][1]
        hi = order[idx + 1][1] if idx + 1 < len(order) else prof["bkt_entry_cnt"]
        path = os.path.join(dst, s["bkt_bin"])
        bkt = np.fromfile(path, dtype=np.float32).reshape(-1, 8).copy()
        for b in range(lo, hi):
            d0, d1, _, _, x0 = bkt[b, :5]
            if x0 <= _ZERO_BELOW:
                bkt[b, 0:4] = 0.0  # masked region: exp -> exact 0
                continue
            if not (d0 > 0 and np.isfinite(d0) and abs(d1 - d0) <= 1e-3 * d0):
                continue  # saturation buckets (inf / 0)
            if x0 > 0:
                # positive side: exp(x - C)
                g = np.float32(np.exp(np.float64(x0) - C_SHIFT))
                bkt[b, 0] = g
                bkt[b, 1] = g
            else:
                # negative side: exp(alpha*x - C) (nearly flat; linear spline)
                g = np.float32(np.exp(_FUSED_ALPHA * np.float64(x0) - C_SHIFT))
                bkt[b, 0] = g
                bkt[b, 1] = np.float32(_FUSED_ALPHA * g)
            bkt[b, 2] = np.float32(0.0)  # cubic terms fault the engine
            bkt[b, 3] = np.float32(0.0)
        bkt.tofile(path)
    return os.path.join(dst, "act_info.json")


def build_nc():
    global _CACHED
    if _CACHED is not None:
        return _CACHED
    os.environ["BASS_ACT_ROOT_JSON_PATH"] = _make_fused_act_root()
    nc = bacc.Bacc("TRN2", target_bir_lowering=False, debug=False,
                   enable_asserts=False, num_devices=NCORES)
    adjT8 = nc.dram_tensor("adjT8", [N, ROWS], FP8, kind="ExternalInput").ap()
    whi = nc.dram_tensor("whi", [N, D], FP8, kind="ExternalInput").ap()
    wlo = nc.dram_tensor("wlo", [N, D], FP8, kind="ExternalInput").ap()
    xT = nc.dram_tensor("xT", [D, N], BF16, kind="ExternalInput").ap()
    a8t = nc.dram_tensor("a8t", [P, 8], BF16, kind="ExternalInput").ap()
    out = nc.dram_tensor("out", [ROWS, D], BF16, kind="ExternalOutput").ap()

    from contextlib import ExitStack
    with tile.TileContext(nc) as tc:
        with ExitStack() as ctx:
            _build_kernel(nc, tc, adjT8, whi, wlo, xT, a8t, out, ctx)
    nc.compile()
    _CACHED = nc
    return nc


def make_in_maps(input, adj_matrix, a):
    E4 = ml_dtypes.float8_e4m3
    BF = ml_dtypes.bfloat16
    x16 = np.asarray(input, dtype=np.float32).astype(BF)
    x16f = x16.astype(np.float32)
    w_hi = np.ascontiguousarray(x16f.astype(E4))
    w_lo = np.ascontiguousarray((x16f - w_hi.astype(np.float32)).astype(E4))
    adj = np.asarray(adj_matrix)
    a_f = np.asarray(a, dtype=np.float32).reshape(-1)
    a8t = np.ascontiguousarray(a_f.reshape(8, P).T.astype(BF))  # [128, 8]
    xT_full = np.ascontiguousarray(x16.T)                        # [D, N] bf16
    in_maps = []
    for c in range(NCORES):
        rows = slice(c * ROWS, (c + 1) * ROWS)
        # per-core j-rotation: tile 0 is always this core's own rows
        rot = np.roll(np.arange(N), -c * ROWS)
        adjT_c = adj[rows, :].T[rot]          # [N(j rotated), ROWS(i local)]
        adjT8_c = np.ascontiguousarray(
            ((adjT_c.astype(np.float32) - 1.0) * (-MASK_NEG)).astype(E4))
        in_maps.append({
            "adjT8": adjT8_c,
            "whi": np.ascontiguousarray(w_hi[rot]),
            "wlo": np.ascontiguousarray(w_lo[rot]),
            "xT": np.ascontiguousarray(xT_full[:, rot]),
            "a8t": a8t,
        })
    return in_maps


def kernel(input, adj_matrix, a, _trace=False, _tmpdir=None):
    nc = build_nc()
    in_maps = make_in_maps(input, adj_matrix, a)
    try:
        res = run_bass_kernel_spmd(nc, in_maps, core_ids=list(range(NCORES)),
                                   trace=_trace, tmpdir=_tmpdir)
    except ModuleNotFoundError:
        res = run_bass_kernel_spmd(nc, in_maps, core_ids=list(range(NCORES)))
    out = np.concatenate(
        [res.results[c]["out"].astype(np.float32) for c in range(NCORES)],
        axis=0)
    kernel._last_results = res
    return out
